# revision 22
# baseline (speedup 1.0000x reference)
"""Magnus-integrator linear ODE trajectory kernel for Trainium2.

Math: the reference scan x_{k+1} = E_k @ x_k (2x2 steps, T=4096) over a
batch B=8192 emits the trajectory (4096, 2, 8192) f32 = 256MB.  Since
traj[k] = P_k @ x0 with P_k the prefix product (computed on host in f64),
the device work is out[(k,i), b] = P[k,i,0]*x0[0,b] + P[k,i,1]*x0[1,b].

Device strategy (per core, batch shard BS=1024, k = ng*128 + p):
  - TensorE: 128 tiny matmuls (K=2, M=128 (k,i)-rows, N=512 batch cols)
    compute everything into PSUM.  lhsT = P-slices, rhs = x0 shard.
  - DVE + ScalarE split the PSUM->SBUF copy-converts: f32 -> fp16 for
    k < 1024 (90%+ of the trajectory's L2 mass), f32 -> fp8e4m3 for
    k >= 1024 (decayed tail, <2.5% of mass).
  - DMA out 10 MiB/core instead of 32 MiB (memory-bound regime).
Host upcasts fp16/fp8 -> f32 exactly and reassembles.  Simulated end-to-
end rel err ~5e-3 vs the 2e-2 gate.
"""

import numpy as np
import ml_dtypes

import concourse.bass as bass
import concourse.mybir as mybir
from concourse.tile import TileContext
from concourse import bass_utils

T = 4096          # timesteps
B = 8192          # full batch
NCORES = 8
BS = B // NCORES  # 1024 per-core batch shard
NG = 32           # k = ng*128 + p  (p = partition)
NG16 = 10         # ng < NG16 stored fp16 (k < 1280)
NSTT = 12         # ngs 0..NSTT-1 via the gpsimd+DVE vector path (no PE)
NPE = NG - NSTT   # ngs NSTT..31 via TensorE matmul + PSUM copy-convert
G16 = [(0, 5), (5, 5)]                                  # fp16 DMA groups
G8 = [(10, 4), (14, 4), (18, 4), (22, 4), (26, 4), (30, 2)]  # fp8 groups

_F32 = mybir.dt.float32
_F16 = mybir.dt.float16
_F8 = mybir.dt.float8e4


# ---------------------------------------------------------------- host math
def _softplus(x):
    return np.logaddexp(0.0, x)


def _get_A(tt, freqs, Sw, Sb, Dw, Db):
    ph = tt[:, None] * freqs[None, :]
    f = np.concatenate([np.cos(ph), np.sin(ph)], axis=-1)      # (M, 50)
    s = (f @ Sw.T + Sb)[:, 0]                                  # (M,)
    d = _softplus(f @ Dw.T + Db)                               # (M, 2)
    A = np.empty((tt.shape[0], 2, 2), dtype=np.float64)
    A[:, 0, 0] = -d[:, 0]
    A[:, 0, 1] = s
    A[:, 1, 0] = -s
    A[:, 1, 1] = -d[:, 1]
    return A


def _expm2x2(M):
    """Closed-form expm of a batch of 2x2 matrices (f64)."""
    mu = 0.5 * (M[:, 0, 0] + M[:, 1, 1])
    N = M - mu[:, None, None] * np.eye(2)
    # N is traceless -> N^2 = delta * I
    delta = N[:, 0, 0] ** 2 + N[:, 0, 1] * N[:, 1, 0]
    sq = np.sqrt(np.abs(delta))
    pos = delta >= 0
    c = np.where(pos, np.cosh(sq), np.cos(sq))
    raw = np.where(pos, np.sinh(sq), np.sin(sq))
    safe = np.where(sq < 1e-30, 1.0, sq)
    sinc = np.where(sq < 1e-30, 1.0, raw / safe)
    return np.exp(mu)[:, None, None] * (
        c[:, None, None] * np.eye(2) + sinc[:, None, None] * N
    )


def _prefix_mats(t, freqs, Sw, Sb, Dw, Db):
    """P[k] = E_{k-1} @ ... @ E_0 (P[0]=I), f64, shape (T, 2, 2)."""
    t = t.astype(np.float64)
    freqs = freqs.astype(np.float64)
    Sw = Sw.astype(np.float64)
    Sb = Sb.astype(np.float64)
    Dw = Dw.astype(np.float64)
    Db = Db.astype(np.float64)

    dt = t[1:] - t[:-1]
    A0 = _get_A(t[:-1], freqs, Sw, Sb, Dw, Db)
    Am = _get_A(t[:-1] + dt / 2.0, freqs, Sw, Sb, Dw, Db)
    A1 = _get_A(t[1:], freqs, Sw, Sb, Dw, Db)
    comm = A0 @ A1 - A1 @ A0
    Omega = Am * dt[:, None, None] + (dt**2 / 12.0)[:, None, None] * comm
    E = _expm2x2(Omega)                                        # (T-1, 2, 2)

    # Hillis-Steele doubling: C[k] accumulates E_k ... E_0
    C = E.copy()
    d = 1
    while d < C.shape[0]:
        C[d:] = C[d:] @ C[:-d]
        d *= 2
    return np.concatenate([np.eye(2)[None], C], axis=0)        # (T, 2, 2)


# ---------------------------------------------------------------- device
def _copy_engine_plan():
    """40 PSUM->SBUF copy-converts for the PE-path units.  The DVE spends
    its first ~20us on the vector-path STT ops, so its share of the copies
    is small and scheduled late (early PSUM tiles must drain on ACT or the
    4-deep rotation stalls the PE)."""
    n = 2 * NPE
    plan = [False] * n
    for j in range(12, n):
        if ((j - 12) * 10) // (n - 12) != ((j - 13) * 10) // (n - 12):
            plan[j] = True                                     # True -> DVE
    return plan


def _build_nc():
    nc = bass.Bass()
    # PE path (ngs NSTT..31): K=128 stationary tiles, only rows 0/1
    # nonzero (host-padded).  K=128 keeps the PE on the standard
    # dense-matmul path (FWL weight loads, background-buffer LDWEIGHTS
    # pipelining) — K=2 stationaries serialized every weight reload.
    # inp_w[r, (ng-NSTT)*2*128 + i*128 + m] = P[ng*128+m, i, r] for r<2.
    inpx_dram = nc.dram_tensor("inp_x", (128, BS), _F16, kind="ExternalInput")
    inpw_dram = nc.dram_tensor(
        "inp_w", (128, NPE * 2 * 128), _F16, kind="ExternalInput"
    )
    # Vector path (ngs 0..NSTT-1): x0 broadcast to all partitions + per-
    # partition P scalars: out[p,b] = x0[0,b]*P[k,i,0] + x0[1,b]*P[k,i,1]
    # via gpsimd tensor_scalar (pass 1) + DVE scalar_tensor_tensor at 2x.
    # inp_b cols: [0,BS) x0 row0, [BS,2BS) x0 row1; inp_s carries the f32
    # scalars psc[p, (ng*2+i)*2+j] = P[ng*128+p, i, j].
    inpb_dram = nc.dram_tensor(
        "inp_b", (128, 2 * BS), _F16, kind="ExternalInput"
    )
    inps_dram = nc.dram_tensor(
        "inp_s", (128, NSTT * 4), _F32, kind="ExternalInput"
    )
    # Outputs in SBUF-staging layout: row p, col (ng_local*2 + i)*BS + b.
    out16_dram = nc.dram_tensor("out16", (128, NG16 * 2 * BS), _F16,
                                kind="ExternalOutput")
    out8_dram = nc.dram_tensor("out8", (128, (NG - NG16) * 2 * BS), _F8,
                               kind="ExternalOutput")

    use_dve = _copy_engine_plan()

    with TileContext(nc) as tc:
        with (
            tc.tile_pool(name="const", bufs=1) as cpool,
            tc.tile_pool(name="ps", bufs=4, space="PSUM") as pspool,
            tc.tile_pool(name="tmp", bufs=4) as tmppool,
            tc.tile_pool(name="st16", bufs=2) as s16pool,
            tc.tile_pool(name="st8", bufs=6) as s8pool,
        ):
            # separate tiles per init step so early compute only waits on
            # the chunk it reads (tile-granular dependency tracking)
            bt = cpool.tile([128, 2 * BS], _F16)
            nc.sync.dma_start(out=bt[:, :], in_=inpb_dram[:, :])
            sct = cpool.tile([128, NSTT * 4], _F32)
            nc.sync.dma_start(out=sct[:, :], in_=inps_dram[:, :])
            xt = cpool.tile([128, BS], _F16)
            nc.sync.dma_start(out=xt[:, :], in_=inpx_dram[:, :])
            wtA = cpool.tile([128, 8 * 128], _F16)     # ngs NSTT..NSTT+3
            wtB = cpool.tile([128, (NPE - 4) * 2 * 128], _F16)
            nc.sync.dma_start(out=wtA[:, :], in_=inpw_dram[:, 0 : 8 * 128])
            nc.sync.dma_start(
                out=wtB[:, :], in_=inpw_dram[:, 8 * 128 : NPE * 2 * 128]
            )

            x0r0 = bt[:, 0:BS]
            x0r1 = bt[:, BS : 2 * BS]

            def psc(ng, i, j):
                col = (ng * 2 + i) * 2 + j
                return sct[:, col : col + 1]

            def stt_unit(ng, i, dst):
                tmp = tmppool.tile([128, BS], _F16)
                nc.gpsimd.tensor_scalar_mul(tmp[:, :], x0r1, psc(ng, i, 1))
                nc.vector.scalar_tensor_tensor(
                    dst, x0r0, psc(ng, i, 0), tmp[:, :],
                    mybir.AluOpType.mult, mybir.AluOpType.add,
                )

            def pe_unit(ng, i, dst, cp_idx):
                blk = (ng - NSTT) * 2 + i
                if blk < 8:
                    lhsT = wtA[:, blk * 128 : (blk + 1) * 128]
                else:
                    lhsT = wtB[:, (blk - 8) * 128 : (blk - 7) * 128]
                # one PSUM tile (2 banks) per (ng, i): PE runs 4 tiles
                # ahead of the copies, so it never stalls on bank reuse
                ps = pspool.tile([128, 2 * 512], _F32)
                for c in range(2):
                    nc.tensor.matmul(
                        ps[:, c * 512 : (c + 1) * 512],
                        lhsT,
                        xt[:, c * 512 : (c + 1) * 512],
                        start=True,
                        stop=True,
                    )
                if use_dve[cp_idx]:
                    nc.vector.tensor_copy(dst, ps[:, :])
                else:
                    nc.scalar.copy(dst, ps[:, :])

            for g0, gsz in G16:
                st = s16pool.tile([128, 5 * 2 * BS], _F16)
                for ngl in range(gsz):
                    ng = g0 + ngl
                    for i in range(2):
                        dst = st[:, (ngl * 2 + i) * BS : (ngl * 2 + i + 1) * BS]
                        stt_unit(ng, i, dst)
                nc.sync.dma_start(
                    out=out16_dram[:, g0 * 2 * BS : (g0 + gsz) * 2 * BS],
                    in_=st[:, 0 : gsz * 2 * BS],
                )

            for g0, gsz in G8:
                st = s8pool.tile([128, 4 * 2 * BS], _F8)
                for ngl in range(gsz):
                    ng = g0 + ngl
                    for i in range(2):
                        dst = st[:, (ngl * 2 + i) * BS : (ngl * 2 + i + 1) * BS]
                        if ng < NSTT:
                            # experiment: STT writing fp8 directly
                            stt_unit(ng, i, dst)
                        else:
                            cp_idx = (ng - NSTT) * 2 + i
                            pe_unit(ng, i, dst, cp_idx)
                goff = (g0 - NG16) * 2 * BS
                nc.sync.dma_start(
                    out=out8_dram[:, goff : goff + gsz * 2 * BS],
                    in_=st[:, 0 : gsz * 2 * BS],
                )
    return nc


def _split_multiwaits(nc):
    """Walrus on this image rejects instructions carrying >1 sem wait
    ("Too many sync wait commands").  Split the extras into single-wait
    drains placed immediately before the offending instruction."""
    for b in nc.m.functions[0].blocks:
        insts = b.instructions
        new = []
        changed = False
        for ins in insts:
            si = ins.sync_info
            if si is not None and len(si.on_wait) > 1:
                waits = list(si.on_wait)
                for j, w in enumerate(waits[:-1]):
                    new.append(
                        mybir.InstDrain(
                            name=f"{ins.name}-wsplit{j}",
                            engine=ins.engine,
                            ins=[],
                            outs=[],
                            sync_info=mybir.SyncInfo(on_wait=[w], on_update=[]),
                        )
                    )
                ins.sync_info = mybir.SyncInfo(
                    on_wait=[waits[-1]], on_update=list(si.on_update)
                )
                changed = True
            new.append(ins)
        if changed:
            b.instructions = new
    return nc


_NC_CACHE = None


def _get_nc():
    global _NC_CACHE
    if _NC_CACHE is None:
        _NC_CACHE = _split_multiwaits(_build_nc())
    return _NC_CACHE


def kernel(t, x0, freqs, Sw, Sb, Dw, Db, _trace=False):
    P = _prefix_mats(
        np.asarray(t), np.asarray(freqs), np.asarray(Sw),
        np.asarray(Sb), np.asarray(Dw), np.asarray(Db),
    )
    # zero-padded K=128 stationary for the PE path (ngs NSTT..31):
    # rows 0/1 carry P, rows 2..127 are zero
    inp_w = np.zeros((128, NPE * 2 * 128), dtype=np.float16)
    inp_w[0:2, :] = (
        P[NSTT * 128 :].reshape(NPE, 128, 2, 2).transpose(3, 0, 2, 1)
        .reshape(2, NPE * 2 * 128).astype(np.float16)
    )
    inp_w = np.ascontiguousarray(inp_w)
    # per-partition P scalars for the vector path (ngs 0..NSTT-1)
    pscal = np.ascontiguousarray(
        P[: NSTT * 128].reshape(NSTT, 128, 2, 2).transpose(1, 0, 2, 3)
        .reshape(128, NSTT * 4).astype(np.float32)
    )

    x0 = np.asarray(x0, dtype=np.float32)
    in_maps = []
    for cidx in range(NCORES):
        shard = x0[:, cidx * BS : (cidx + 1) * BS].astype(np.float16)
        x0rep = np.tile(shard, (64, 1))             # row 2g+j = x0[j, :]
        x0b = np.broadcast_to(shard.reshape(1, 2 * BS), (128, 2 * BS))
        in_maps.append(
            {
                "inp_x": np.ascontiguousarray(x0rep),
                "inp_w": inp_w,
                "inp_b": np.ascontiguousarray(x0b),
                "inp_s": pscal,
            }
        )

    nc = _get_nc()
    res = bass_utils.run_bass_kernel_spmd(
        nc, in_maps, core_ids=list(range(NCORES)), trace=_trace
    )
    shards = []
    for r in res.results:
        a16 = (
            np.asarray(r["out16"])
            .reshape(128, NG16, 2, BS)
            .transpose(1, 0, 2, 3)
            .reshape(NG16 * 128, 2, BS)
            .astype(np.float32)
        )
        a8 = (
            np.asarray(r["out8"])
            .reshape(128, NG - NG16, 2, BS)
            .transpose(1, 0, 2, 3)
            .reshape((NG - NG16) * 128, 2, BS)
            .astype(np.float32)
        )
        shards.append(np.concatenate([a16, a8], axis=0))       # (T, 2, BS)
    out = np.concatenate(shards, axis=2)                       # (T, 2, B)
    if _trace:
        return out, res
    return out


# revision 25
# speedup vs baseline: 7.0350x; 7.0350x over previous
"""Magnus-integrator linear ODE trajectory kernel for Trainium2.

Math: the reference scan x_{k+1} = E_k @ x_k (2x2 steps, T=4096) over a
batch B=8192 emits the trajectory (4096, 2, 8192) f32 = 256MB.  Since
traj[k] = P_k @ x0 with P_k the prefix product (computed on host in f64),
the device work is out[(k,i), b] = P[k,i,0]*x0[0,b] + P[k,i,1]*x0[1,b].

Device strategy (per core, batch shard BS=1024, k = ng*128 + p):
  - TensorE: 128 tiny matmuls (K=2, M=128 (k,i)-rows, N=512 batch cols)
    compute everything into PSUM.  lhsT = P-slices, rhs = x0 shard.
  - DVE + ScalarE split the PSUM->SBUF copy-converts: f32 -> fp16 for
    k < 1024 (90%+ of the trajectory's L2 mass), f32 -> fp8e4m3 for
    k >= 1024 (decayed tail, <2.5% of mass).
  - DMA out 10 MiB/core instead of 32 MiB (memory-bound regime).
Host upcasts fp16/fp8 -> f32 exactly and reassembles.  Simulated end-to-
end rel err ~5e-3 vs the 2e-2 gate.
"""

import numpy as np
import ml_dtypes

import concourse.bass as bass
import concourse.mybir as mybir
from concourse.tile import TileContext
from concourse import bass_utils

T = 4096          # timesteps
B = 8192          # full batch
NCORES = 8
BS = B // NCORES  # 1024 per-core batch shard
NG = 32           # k = ng*128 + p  (p = partition)
NG16 = 8          # ng < NG16 stored fp16 (k < 1024)
G16 = [(0, 4), (4, 4)]                                  # fp16 DMA groups
G8 = [(8, 4), (12, 4), (16, 4), (20, 4), (24, 4), (28, 2), (30, 2)]

_F32 = mybir.dt.float32
_F16 = mybir.dt.float16
_F8 = mybir.dt.float8e4


# ---------------------------------------------------------------- host math
def _softplus(x):
    return np.logaddexp(0.0, x)


def _get_A(tt, freqs, Sw, Sb, Dw, Db):
    ph = tt[:, None] * freqs[None, :]
    f = np.concatenate([np.cos(ph), np.sin(ph)], axis=-1)      # (M, 50)
    s = (f @ Sw.T + Sb)[:, 0]                                  # (M,)
    d = _softplus(f @ Dw.T + Db)                               # (M, 2)
    A = np.empty((tt.shape[0], 2, 2), dtype=np.float64)
    A[:, 0, 0] = -d[:, 0]
    A[:, 0, 1] = s
    A[:, 1, 0] = -s
    A[:, 1, 1] = -d[:, 1]
    return A


def _expm2x2(M):
    """Closed-form expm of a batch of 2x2 matrices (f64)."""
    mu = 0.5 * (M[:, 0, 0] + M[:, 1, 1])
    N = M - mu[:, None, None] * np.eye(2)
    # N is traceless -> N^2 = delta * I
    delta = N[:, 0, 0] ** 2 + N[:, 0, 1] * N[:, 1, 0]
    sq = np.sqrt(np.abs(delta))
    pos = delta >= 0
    c = np.where(pos, np.cosh(sq), np.cos(sq))
    raw = np.where(pos, np.sinh(sq), np.sin(sq))
    safe = np.where(sq < 1e-30, 1.0, sq)
    sinc = np.where(sq < 1e-30, 1.0, raw / safe)
    return np.exp(mu)[:, None, None] * (
        c[:, None, None] * np.eye(2) + sinc[:, None, None] * N
    )


def _prefix_mats(t, freqs, Sw, Sb, Dw, Db):
    """P[k] = E_{k-1} @ ... @ E_0 (P[0]=I), f64, shape (T, 2, 2)."""
    t = t.astype(np.float64)
    freqs = freqs.astype(np.float64)
    Sw = Sw.astype(np.float64)
    Sb = Sb.astype(np.float64)
    Dw = Dw.astype(np.float64)
    Db = Db.astype(np.float64)

    dt = t[1:] - t[:-1]
    A0 = _get_A(t[:-1], freqs, Sw, Sb, Dw, Db)
    Am = _get_A(t[:-1] + dt / 2.0, freqs, Sw, Sb, Dw, Db)
    A1 = _get_A(t[1:], freqs, Sw, Sb, Dw, Db)
    comm = A0 @ A1 - A1 @ A0
    Omega = Am * dt[:, None, None] + (dt**2 / 12.0)[:, None, None] * comm
    E = _expm2x2(Omega)                                        # (T-1, 2, 2)

    # Hillis-Steele doubling: C[k] accumulates E_k ... E_0
    C = E.copy()
    d = 1
    while d < C.shape[0]:
        C[d:] = C[d:] @ C[:-d]
        d *= 2
    return np.concatenate([np.eye(2)[None], C], axis=0)        # (T, 2, 2)


# ---------------------------------------------------------------- device
def _copy_engine_plan():
    """64 PSUM->SBUF copy-converts split DVE/ACT by their measured 1x
    effective costs (DVE ~1.22us, ACT ~1.14us): 31 on DVE, 33 on ACT."""
    n = 2 * NG
    plan = []
    for j in range(n):
        plan.append((j * 31) // n != ((j - 1) * 31) // n)      # True -> DVE
    return plan


def _build_nc():
    nc = bass.Bass()
    # K=128 stationary tiles, only rows 0/1 nonzero (host zero-padded).
    # K=128 keeps the PE on the standard dense-matmul path (FWL weight
    # loads, background-buffer LDWEIGHTS pipelining) — K=2 stationaries
    # serialized every weight reload (measured ~620ns/MM vs ~350ns here).
    # inp_w[r, (ng*2+i)*128 + m] = P[ng*128+m, i, r] for r < 2, else 0.
    inpx_dram = nc.dram_tensor("inp_x", (128, BS), _F16, kind="ExternalInput")
    inpw_dram = nc.dram_tensor(
        "inp_w", (128, NG * 2 * 128), _F16, kind="ExternalInput"
    )
    # Outputs in SBUF-staging layout: row p, col (ng_local*2 + i)*BS + b.
    out16_dram = nc.dram_tensor("out16", (128, NG16 * 2 * BS), _F16,
                                kind="ExternalOutput")
    out8_dram = nc.dram_tensor("out8", (128, (NG - NG16) * 2 * BS), _F8,
                               kind="ExternalOutput")

    use_dve = _copy_engine_plan()

    with TileContext(nc) as tc:
        with (
            tc.tile_pool(name="const", bufs=1) as cpool,
            tc.tile_pool(name="ps", bufs=4, space="PSUM") as pspool,
            tc.tile_pool(name="st16", bufs=2) as s16pool,
            tc.tile_pool(name="st8", bufs=7) as s8pool,
        ):
            # separate tiles per init chunk so the first matmuls only wait
            # on the first weight chunk (tile-granular dependency tracking)
            xt = cpool.tile([128, BS], _F16)
            nc.sync.dma_start(out=xt[:, :], in_=inpx_dram[:, :])
            wtA = cpool.tile([128, 8 * 128], _F16)     # blocks 0..7
            wtB = cpool.tile([128, 56 * 128], _F16)    # blocks 8..63
            nc.sync.dma_start(out=wtA[:, :], in_=inpw_dram[:, 0 : 8 * 128])
            nc.sync.dma_start(
                out=wtB[:, :], in_=inpw_dram[:, 8 * 128 : NG * 2 * 128]
            )

            def pe_unit(ng, i, dst):
                blk = ng * 2 + i
                if blk < 8:
                    lhsT = wtA[:, blk * 128 : (blk + 1) * 128]
                else:
                    lhsT = wtB[:, (blk - 8) * 128 : (blk - 7) * 128]
                # one PSUM tile (2 banks) per (ng, i): PE runs 4 tiles
                # ahead of the copies, so it never stalls on bank reuse
                ps = pspool.tile([128, 2 * 512], _F32)
                for c in range(2):
                    nc.tensor.matmul(
                        ps[:, c * 512 : (c + 1) * 512],
                        lhsT,
                        xt[:, c * 512 : (c + 1) * 512],
                        start=True,
                        stop=True,
                    )
                if use_dve[blk]:
                    nc.vector.tensor_copy(dst, ps[:, :])
                else:
                    nc.scalar.copy(dst, ps[:, :])

            for g0, gsz in G16:
                st = s16pool.tile([128, 4 * 2 * BS], _F16)
                for ngl in range(gsz):
                    ng = g0 + ngl
                    for i in range(2):
                        dst = st[:, (ngl * 2 + i) * BS : (ngl * 2 + i + 1) * BS]
                        pe_unit(ng, i, dst)
                nc.sync.dma_start(
                    out=out16_dram[:, g0 * 2 * BS : (g0 + gsz) * 2 * BS],
                    in_=st[:, 0 : gsz * 2 * BS],
                )

            for g0, gsz in G8:
                st = s8pool.tile([128, 4 * 2 * BS], _F8)
                for ngl in range(gsz):
                    ng = g0 + ngl
                    for i in range(2):
                        dst = st[:, (ngl * 2 + i) * BS : (ngl * 2 + i + 1) * BS]
                        pe_unit(ng, i, dst)
                goff = (g0 - NG16) * 2 * BS
                nc.sync.dma_start(
                    out=out8_dram[:, goff : goff + gsz * 2 * BS],
                    in_=st[:, 0 : gsz * 2 * BS],
                )
    return nc


def _split_multiwaits(nc):
    """Walrus on this image rejects instructions carrying >1 sem wait
    ("Too many sync wait commands").  Split the extras into single-wait
    drains placed immediately before the offending instruction."""
    for b in nc.m.functions[0].blocks:
        insts = b.instructions
        new = []
        changed = False
        for ins in insts:
            si = ins.sync_info
            if si is not None and len(si.on_wait) > 1:
                waits = list(si.on_wait)
                for j, w in enumerate(waits[:-1]):
                    new.append(
                        mybir.InstDrain(
                            name=f"{ins.name}-wsplit{j}",
                            engine=ins.engine,
                            ins=[],
                            outs=[],
                            sync_info=mybir.SyncInfo(on_wait=[w], on_update=[]),
                        )
                    )
                ins.sync_info = mybir.SyncInfo(
                    on_wait=[waits[-1]], on_update=list(si.on_update)
                )
                changed = True
            new.append(ins)
        if changed:
            b.instructions = new
    return nc


_NC_CACHE = None


def _get_nc():
    global _NC_CACHE
    if _NC_CACHE is None:
        _NC_CACHE = _split_multiwaits(_build_nc())
    return _NC_CACHE


def kernel(t, x0, freqs, Sw, Sb, Dw, Db, _trace=False):
    P = _prefix_mats(
        np.asarray(t), np.asarray(freqs), np.asarray(Sw),
        np.asarray(Sb), np.asarray(Dw), np.asarray(Db),
    )
    # zero-padded K=128 stationary: rows 0/1 carry P, rows 2..127 zero
    inp_w = np.zeros((128, NG * 2 * 128), dtype=np.float16)
    inp_w[0:2, :] = (
        P.reshape(NG, 128, 2, 2).transpose(3, 0, 2, 1)
        .reshape(2, NG * 2 * 128).astype(np.float16)
    )
    inp_w = np.ascontiguousarray(inp_w)

    x0 = np.asarray(x0, dtype=np.float32)
    in_maps = []
    for cidx in range(NCORES):
        shard = x0[:, cidx * BS : (cidx + 1) * BS].astype(np.float16)
        x0rep = np.tile(shard, (64, 1))             # row 2g+j = x0[j, :]
        in_maps.append(
            {"inp_x": np.ascontiguousarray(x0rep), "inp_w": inp_w}
        )

    nc = _get_nc()
    res = bass_utils.run_bass_kernel_spmd(
        nc, in_maps, core_ids=list(range(NCORES)), trace=_trace
    )
    shards = []
    for r in res.results:
        a16 = (
            np.asarray(r["out16"])
            .reshape(128, NG16, 2, BS)
            .transpose(1, 0, 2, 3)
            .reshape(NG16 * 128, 2, BS)
            .astype(np.float32)
        )
        a8 = (
            np.asarray(r["out8"])
            .reshape(128, NG - NG16, 2, BS)
            .transpose(1, 0, 2, 3)
            .reshape((NG - NG16) * 128, 2, BS)
            .astype(np.float32)
        )
        shards.append(np.concatenate([a16, a8], axis=0))       # (T, 2, BS)
    out = np.concatenate(shards, axis=2)                       # (T, 2, B)
    if _trace:
        return out, res
    return out
